# revision 1
# baseline (speedup 1.0000x reference)
"""Trainium2 Bass kernel for CrossAttention (B=4, N=M=2048, H=8, D=64,
Q_DIM=1024, C_DIM=768).

Sharding over 8 cores: core c handles batch b = c//2 and head-group
hg = c%2 (4 heads, 256 inner dims).  Each core computes a *partial*
output projection (its 256 inner dims of the 512 total); the host sums
core pairs and adds the output bias.

Device-side layouts are all matmul-native (out = lhsT.T @ rhs):
  - qT/kT [inner, seq]: computed with weight-chunk stationary, xT/ctxT
    moving.  v [keys, inner] with ctxT-chunk stationary, WvT moving.
  - scores are computed transposed: S.T[keys, q] = kT_h contracted with
    qT_h over the head dim, so softmax's key-reduction is a matmul
    reduction and no on-chip transposes are needed anywhere.
  - V carries an appended ones-column, so the P.T@V matmul also yields the
    per-query softmax denominators (row 64 of the [65, q] accumulator).
  - denominators are broadcast across partitions with a K=1 ones matmul,
    reciprocal'd on VectorE, and folded into the PSUM->SBUF copy of the
    attention output.
  - all matmul inputs are float32r (single-pass FP22 multiply, fp32
    accumulate) for 4x tensor-engine throughput vs true fp32.
  - the attention inner loop is software-pipelined at emission so the
    next chunk's score matmuls sit ahead of the current chunk's PV
    matmul in the PE queue (PV waits on exp; the scores don't).

The attention mask in this problem is all-True; if a mask with False
entries is ever passed, kernel() falls back to a numpy reference.
"""

import numpy as np

B, N, M = 4, 2048, 2048
Q_DIM, C_DIM, H, D = 1024, 768, 8, 64
INNER = H * D  # 512
SCALE = D ** -0.5

N_CORES = 8
H_PER_CORE = 4          # heads per core
IN_PER_CORE = H_PER_CORE * D  # 256 inner dims per core
QB = 1024               # query block
N_QB = N // QB          # 2
KC = M // 128           # 16 key chunks
QK_CHUNKS = Q_DIM // 128   # 8
CK_CHUNKS = C_DIM // 128   # 6
IN_CHUNKS = IN_PER_CORE // 128  # 2

_CACHED_NC = None


def _build_bass():
    import concourse.bass as bass
    import concourse.mybir as mybir
    import concourse.tile as tile
    from concourse import bacc

    f32r = mybir.dt.float32r
    f32 = mybir.dt.float32
    ts, ds = bass.ts, bass.ds
    Exp = mybir.ActivationFunctionType.Exp

    nc = bacc.Bacc("TRN2", target_bir_lowering=False)

    xT = nc.dram_tensor("xT", [Q_DIM, N], f32r, kind="ExternalInput")
    cT = nc.dram_tensor("cT", [C_DIM, M], f32r, kind="ExternalInput")
    wq = nc.dram_tensor("wq", [Q_DIM, IN_PER_CORE], f32r, kind="ExternalInput")
    wk = nc.dram_tensor("wk", [C_DIM, IN_PER_CORE], f32r, kind="ExternalInput")
    wv = nc.dram_tensor("wv", [C_DIM, IN_PER_CORE], f32r, kind="ExternalInput")
    wo = nc.dram_tensor("wo", [IN_PER_CORE, Q_DIM], f32r, kind="ExternalInput")
    out_d = nc.dram_tensor("out", [N, Q_DIM], f32, kind="ExternalOutput")

    with tile.TileContext(nc) as tc:
        with (
            tc.tile_pool(name="persist", bufs=1) as persist,
            tc.tile_pool(name="stream", bufs=2) as stream,
            tc.tile_pool(name="psA", bufs=2, space="PSUM") as psA,
            tc.tile_pool(name="psB", bufs=2, space="PSUM") as psB,
        ):
            # ---- resident weights / constants ----
            # DMA order matters: the kT phase needs wk + ctx first.
            wk_sb = persist.tile([128, CK_CHUNKS, IN_PER_CORE], f32r, tag="wk")
            nc.sync.dma_start(out=wk_sb, in_=wk.rearrange("(k p) n -> p k n", p=128))

            # row D used as the lhsT of the denominator-broadcast matmul; it
            # must sit at the same base partition as the denominator row
            # (partition D of the attention accumulator).  memset can't target
            # float32r, so fill an fp32 scratch and copy-convert.
            onesf = persist.tile([128, D], f32, tag="onesf")
            nc.vector.memset(onesf, 1.0)
            ones_sb = persist.tile([D + 1, D], f32r, tag="ones")
            nc.vector.tensor_copy(out=ones_sb, in_=onesf[0 : D + 1, :])

            # context resident (read by both kT and v phases); per-chunk DMAs
            # so the kT matmuls can start as soon as the first chunk lands.
            ctx_sb = persist.tile([128, CK_CHUNKS, M], f32r, tag="ctx")
            for k in range(CK_CHUNKS):
                nc.sync.dma_start(out=ctx_sb[:, k, :], in_=cT[ds(k * 128, 128), :])

            wq_sb = persist.tile([128, QK_CHUNKS, IN_PER_CORE], f32r, tag="wq")
            wq_r = wq.rearrange("(k p) n -> p k n", p=128)
            nc.sync.dma_start(out=wq_sb[:, :, 0:128], in_=wq_r[:, :, 0:128])

            # load order: x0 right after wq (gates the first exp), then wv
            # (gates the deferred v chunks), wo, and x1 last (needed ~100us in;
            # it waits on the x slot release semaphore at runtime anyway).
            def emit_x_load(qb, chunked=False):
                x_sb = stream.tile([128, QK_CHUNKS, QB], f32r, tag="x", bufs=1,
                                   name=f"x{qb}")
                if chunked:
                    # per-chunk DMAs so the first qT pass paces with arrival
                    for k in range(QK_CHUNKS):
                        nc.sync.dma_start(
                            out=x_sb[:, k, :],
                            in_=xT[ds(k * 128, 128), ds(qb * QB, QB)],
                        )
                else:
                    nc.sync.dma_start(
                        out=x_sb,
                        in_=xT[:, ds(qb * QB, QB)].rearrange("(k p) q -> p k q",
                                                             p=128),
                    )
                return x_sb

            x_tiles = [emit_x_load(0, chunked=True)]

            wv_sb = persist.tile([128, CK_CHUNKS, IN_PER_CORE], f32r, tag="wv")
            nc.sync.dma_start(out=wv_sb, in_=wv.rearrange("(k p) n -> p k n", p=128))
            wo_sb = persist.tile([128, IN_CHUNKS, Q_DIM], f32r, tag="wo")
            nc.sync.dma_start(out=wo_sb, in_=wo.rearrange("(t p) n -> p t n", p=128))
            nc.sync.dma_start(out=wq_sb[:, :, 128:256], in_=wq_r[:, :, 128:256])
            x_tiles.append(emit_x_load(1))

            # ---- kT [IN_PER_CORE, M] interleaved with qT(qb0) so the PE can
            # alternate between them as the ctx / x0 DMAs land (kps and qps
            # occupy the two A slots concurrently).
            kT_sb = persist.tile([128, IN_CHUNKS, M], f32r, tag="kt")
            qT0_sb = stream.tile([128, IN_CHUNKS, QB], f32r, tag="qt", bufs=1,
                                 name="qT0")

            def emit_kt(m, half):
                kps = psA.tile([128, 1024], f32, tag="A", name=f"kps{m}{half}")
                for k in range(CK_CHUNKS):
                    for j in range(2):
                        nc.tensor.matmul(
                            kps[:, ts(j, 512)],
                            wk_sb[:, k, ts(m, 128)],
                            ctx_sb[:, k, ds(half * 1024 + j * 512, 512)],
                            start=(k == 0),
                            stop=(k == CK_CHUNKS - 1),
                        )
                nc.vector.tensor_copy(
                    out=kT_sb[:, m, ds(half * 1024, 1024)], in_=kps
                )

            def emit_qt(qT_sb, x_sb, m, qb, pool=None, tag="A"):
                pool = pool if pool is not None else psA
                qps = pool.tile([128, QB], f32, tag=tag, name=f"qps{qb}{m}")
                for k in range(QK_CHUNKS):
                    for j in range(2):
                        nc.tensor.matmul(
                            qps[:, ts(j, 512)],
                            wq_sb[:, k, ts(m, 128)],
                            x_sb[:, k, ts(j, 512)],
                            start=(k == 0),
                            stop=(k == QK_CHUNKS - 1),
                        )
                nc.vector.tensor_copy(out=qT_sb[:, m, :], in_=qps)

            for m in range(IN_CHUNKS):
                for half in range(2):
                    emit_kt(m, half)
            emit_qt(qT0_sb, x_tiles[0], 0, 0)
            qT1_sb = stream.tile([128, IN_CHUNKS, QB], f32r, tag="qt2", bufs=1,
                                 name="qT1")
            qT_tiles = [qT0_sb, qT1_sb]
            # remaining qT passes are deferred into qb0's attention loop (on
            # the B ring): (h, kc) -> emission closure
            deferred_qt = {
                (0, 6): lambda: emit_qt(qT0_sb, x_tiles[0], 1, 0, psB, "B"),
                (2, 6): lambda: emit_qt(qT1_sb, x_tiles[1], 0, 1, psB, "B"),
                (3, 6): lambda: emit_qt(qT1_sb, x_tiles[1], 1, 1, psB, "B"),
            }

            # ---- v [M, 4, 65]: per key-chunk rows; col 64 of each head = 1.0.
            # The 16 projection chunks are *deferred*: they are emitted inside
            # the first attention head's loop (on the B PSUM ring) where the
            # PE has exp-shadow slack and ctx is fully resident.
            v_sb = persist.tile([128, KC, H_PER_CORE, D + 1], f32r, tag="v")
            nc.vector.tensor_copy(
                out=v_sb[:, :, :, D : D + 1],
                in_=onesf.rearrange("p (a b c) -> p a b c", a=KC, b=H_PER_CORE),
            )

            def emit_v_chunk(kc):
                vps = psB.tile([128, IN_PER_CORE], f32, tag="B", name=f"vps{kc}")
                for k in range(CK_CHUNKS):
                    nc.tensor.matmul(
                        vps,
                        ctx_sb[:, k, ts(kc, 128)],
                        wv_sb[:, k, :],
                        start=(k == 0),
                        stop=(k == CK_CHUNKS - 1),
                    )
                nc.vector.tensor_copy(
                    out=v_sb[:, kc, :, 0:D],
                    in_=vps.rearrange("p (h d) -> p h d", h=H_PER_CORE),
                )

            # ---- per query-block pipeline ----
            # final-projection chunks of the previous block are interleaved
            # into the (ACT-bound) attention loop of the current block so the
            # PE does them in exp shadows instead of an ACT-idle stretch.
            def emit_final_chunk(qb, qm, ot_all, on_act=False):
                ops = psB.tile([128, Q_DIM], f32, tag="B", name=f"ops{qb}{qm}")
                for t in range(IN_CHUNKS):
                    for j in range(2):
                        nc.tensor.matmul(
                            ops[:, ts(j, 512)],
                            ot_all[:, t, ts(qm, 128)],
                            wo_sb[:, t, ts(j, 512)],
                            start=(t == 0),
                            stop=(t == IN_CHUNKS - 1),
                        )
                ost = stream.tile([128, Q_DIM], f32, tag="ost", bufs=2,
                                  name=f"ost{qb}{qm}")
                # tail finals run after the last exp: ScalarE is idle there,
                # VectorE is not (it owns the last normalize chain)
                if on_act:
                    nc.scalar.copy(out=ost, in_=ops)
                else:
                    nc.vector.tensor_copy(out=ost, in_=ops)
                nc.gpsimd.dma_start(
                    out=out_d[ds(qb * QB + qm * 128, 128), :], in_=ost
                )

            prev_final = None  # (qb, ot_all) awaiting final projection
            v_queue = list(range(KC))  # v chunks not yet emitted
            for qb in range(N_QB):
                final_queue = (
                    [(prev_final[0], qm, prev_final[1]) for qm in range(QB // 128)]
                    if prev_final is not None
                    else []
                )

                qT_sb = qT_tiles[qb]

                # attention output (normalized), [128, 2, QB] inner-major
                ot_all = stream.tile([128, IN_CHUNKS, QB], f32r, tag="otall",
                                     bufs=2, name=f"otall{qb}")

                # Software-pipelined attention: emit S(kc) before PV(kc-1) so
                # the PE queue never head-of-line blocks on exp.
                pending = None  # (h, kc, pt, ot_ps)

                def emit_pv(p):
                    h, kc, pt, ot_ps = p
                    for j in range(2):
                        nc.tensor.matmul(
                            ot_ps[:, ts(j, 512)],
                            v_sb[:, kc, h, :],
                            pt[:, ts(j, 512)],
                            start=(kc == 0),
                            stop=(kc == KC - 1),
                        )

                def emit_normalize(h, ot_ps, on_act=False):
                    t, po = h // 2, (h % 2) * D
                    ot_raw = stream.tile([D + 1, QB], f32r, tag="otraw", bufs=2,
                                         name=f"otraw{qb}{h}")
                    # the very last head's normalize gates the tail finals;
                    # ScalarE is idle there while VectorE is the chain itself
                    if on_act:
                        nc.scalar.copy(out=ot_raw, in_=ot_ps)
                    else:
                        nc.vector.tensor_copy(out=ot_raw, in_=ot_ps)
                    bc_ps = psB.tile([D, QB], f32, tag="B", name=f"bc{qb}{h}")
                    for j in range(2):
                        nc.tensor.matmul(
                            bc_ps[:, ts(j, 512)],
                            ones_sb[D : D + 1, :],
                            ot_raw[D : D + 1, ts(j, 512)],
                            start=True,
                            stop=True,
                        )
                    nc.vector.reciprocal(out=bc_ps, in_=bc_ps)
                    nc.vector.tensor_mul(
                        out=ot_all[po : po + D, t, :],
                        in0=ot_raw[0:D, :],
                        in1=bc_ps,
                    )

                for h in range(H_PER_CORE):
                    t, po = h // 2, (h % 2) * D
                    ot_ps = psB.tile([D + 1, QB], f32, tag="B", name=f"ot{qb}{h}")
                    for kc in range(KC):
                        st = psA.tile([128, QB], f32, tag="A", name=f"st{qb}{h}{kc}")
                        for j in range(2):
                            nc.tensor.matmul(
                                st[:, ts(j, 512)],
                                kT_sb[po : po + D, t, ts(kc, 128)],
                                qT_sb[po : po + D, t, ts(j, 512)],
                                start=True,
                                stop=True,
                            )
                        if pending is not None:
                            emit_pv(pending)
                            if pending[1] == KC - 1:  # last chunk of a head
                                emit_normalize(pending[0], pending[3])
                        pt = stream.tile([128, QB], f32r, tag="pt", bufs=3,
                                         name=f"pt{qb}{h}{kc}")
                        nc.scalar.activation(out=pt, in_=st, func=Exp, scale=SCALE)
                        pending = (h, kc, pt, ot_ps)
                        # v chunks stay two key-chunks ahead of the PV stream
                        while v_queue and len(v_queue) > KC - 2 * (kc + 1):
                            emit_v_chunk(v_queue.pop(0))
                        if qb == 0 and (h, kc) in deferred_qt:
                            deferred_qt.pop((h, kc))()
                        # two previous-block final chunks per head, mid-loop
                        if final_queue and kc in (6, 12):
                            emit_final_chunk(*final_queue.pop(0))
                            emit_final_chunk(*final_queue.pop(0))

                # flush the last head of this block
                emit_pv(pending)
                emit_normalize(pending[0], pending[3], on_act=(qb == N_QB - 1))
                pending = None
                for fc in final_queue:
                    emit_final_chunk(*fc)
                prev_final = (qb, ot_all)

            # final projection of the last block
            for qm in range(QB // 128):
                emit_final_chunk(prev_final[0], qm, prev_final[1], on_act=True)

    nc.finalize()
    return nc


def _get_nc():
    global _CACHED_NC
    if _CACHED_NC is None:
        _CACHED_NC = _build_bass()
    return _CACHED_NC


def _numpy_fallback(x, context, mask, Wq, Wk, Wv, Wout, bout):
    q = (x @ Wq.T).reshape(B, N, H, D)
    k = (context @ Wk.T).reshape(B, M, H, D)
    v = (context @ Wv.T).reshape(B, M, H, D)
    sim = np.einsum("bnhd,bmhd->bhnm", q, k) * SCALE
    sim = np.where(mask[:, None, None, :], sim, -np.finfo(np.float32).max)
    sim -= sim.max(axis=-1, keepdims=True)
    attn = np.exp(sim)
    attn /= attn.sum(axis=-1, keepdims=True)
    out = np.einsum("bhnm,bmhd->bnhd", attn, v).reshape(B, N, INNER)
    return (out @ Wout.T + bout).astype(np.float32)


def kernel(x, context, mask, Wq, Wk, Wv, Wout, bout, _want_results=False):
    x = np.asarray(x, dtype=np.float32)
    context = np.asarray(context, dtype=np.float32)
    mask = np.asarray(mask)
    Wq = np.asarray(Wq, dtype=np.float32)
    Wk = np.asarray(Wk, dtype=np.float32)
    Wv = np.asarray(Wv, dtype=np.float32)
    Wout = np.asarray(Wout, dtype=np.float32)
    bout = np.asarray(bout, dtype=np.float32)

    if not mask.all():
        return _numpy_fallback(x, context, mask, Wq, Wk, Wv, Wout, bout)

    from concourse.bass_utils import run_bass_kernel_spmd

    in_maps = []
    for c in range(N_CORES):
        b, hg = c // 2, c % 2
        sl = slice(hg * IN_PER_CORE, (hg + 1) * IN_PER_CORE)
        in_maps.append(
            {
                "xT": np.ascontiguousarray(x[b].T),
                "cT": np.ascontiguousarray(context[b].T),
                "wq": np.ascontiguousarray(Wq[sl, :].T),
                "wk": np.ascontiguousarray(Wk[sl, :].T),
                "wv": np.ascontiguousarray(Wv[sl, :].T),
                "wo": np.ascontiguousarray(Wout[:, sl].T),
            }
        )

    res = run_bass_kernel_spmd(_get_nc(), in_maps, core_ids=list(range(N_CORES)))

    out = np.empty((B, N, Q_DIM), dtype=np.float32)
    for b in range(B):
        out[b] = res.results[2 * b]["out"] + res.results[2 * b + 1]["out"] + bout
    if _want_results:
        return out, res
    return out



# revision 18
# speedup vs baseline: 1.3327x; 1.3327x over previous
"""Trainium2 Bass kernel for CrossAttention (B=4, N=M=2048, H=8, D=64,
Q_DIM=1024, C_DIM=768).

Sharding over 8 cores: core c handles batch b = c//2 and head-group
hg = c%2 (4 heads, 256 inner dims).  Each core computes a *partial*
output projection (its 256 inner dims of the 512 total); the host sums
core pairs and adds the output bias.

v3 design (vs the f32r baseline):
  - everything on-chip is bf16 (inputs converted on host): halves all
    DMA traffic and SBUF footprints at the same tensor-engine rate.
  - scores stay transposed (S.T[keys, q] per (head, key-chunk)) so the
    softmax key-reduction needs no transpose; exp runs on ACT writing
    bf16 probabilities directly.
  - PV is computed *query-major*: out.T[q, d+1] with the probability
    chunk stationary and the (d+1)-wide V chunk moving, so the PE
    streams 65 columns instead of 512 per key-chunk.  The appended
    ones column lands the softmax denominator in the SAME partition as
    its query; normalization is one reciprocal + one free-broadcast
    multiply per 4-query-chunk group on the DVE.
  - PV psums alternate between two 4-chunk tiles so the psum ring
    never creates a false PV->normalize serialization.
  - the normalized [q, d] tiles are transposed back to [d, q] via PE
    identity-transposes (128 cycles each) to feed the final projection.
  - kT/qT projections for the first query block are interleaved with
    the input DMA arrival; everything else rides the S->exp->PV stream
    as sub-microsecond exp-shadow fillers.  The psA psum ring holds
    ONLY score tiles so the exp stream stays strictly double-buffered.

The attention mask in this problem is all-True; if a mask with False
entries is ever passed, kernel() falls back to a numpy reference.
"""

import numpy as np

B, N, M = 4, 2048, 2048
Q_DIM, C_DIM, H, D = 1024, 768, 8, 64
INNER = H * D  # 512
SCALE = D ** -0.5

N_CORES = 8
H_PER_CORE = 4          # heads per core
IN_PER_CORE = H_PER_CORE * D  # 256 inner dims per core
QB = 1024               # query block
N_QB = N // QB          # 2
KC = M // 128           # 16 key chunks
QK_CHUNKS = Q_DIM // 128   # 8
CK_CHUNKS = C_DIM // 128   # 6
IN_CHUNKS = IN_PER_CORE // 128  # 2
QC = QB // 128          # 8 query sub-chunks per block

_CACHED_NC = None


def _build_bass():
    import concourse.bass as bass
    import concourse.mybir as mybir
    import concourse.tile as tile
    from concourse import bacc
    from concourse import masks

    bf16 = mybir.dt.bfloat16
    f32 = mybir.dt.float32
    ts, ds = bass.ts, bass.ds
    Exp = mybir.ActivationFunctionType.Exp

    nc = bacc.Bacc("TRN2", target_bir_lowering=False)

    xT = nc.dram_tensor("xT", [Q_DIM, N], bf16, kind="ExternalInput")
    cT = nc.dram_tensor("cT", [C_DIM, M], bf16, kind="ExternalInput")
    wq = nc.dram_tensor("wq", [Q_DIM, IN_PER_CORE], bf16, kind="ExternalInput")
    wk = nc.dram_tensor("wk", [C_DIM, IN_PER_CORE], bf16, kind="ExternalInput")
    wv = nc.dram_tensor("wv", [C_DIM, IN_PER_CORE], bf16, kind="ExternalInput")
    wo = nc.dram_tensor("wo", [IN_PER_CORE, Q_DIM], bf16, kind="ExternalInput")
    out_d = nc.dram_tensor("out", [N, Q_DIM], bf16, kind="ExternalOutput")

    with tile.TileContext(nc) as tc:
        with (
            tc.tile_pool(name="persist", bufs=1) as persist,
            tc.tile_pool(name="stream", bufs=2) as stream,
            tc.tile_pool(name="psA", bufs=2, space="PSUM") as psA,
            tc.tile_pool(name="pvps", bufs=2, space="PSUM") as pvps,
            tc.tile_pool(name="psB", bufs=2, space="PSUM") as psB,
        ):
            # ---- input DMAs: wk/wq first, then ctx and x0 interleaved so
            # the kT chain and the qT chain complete together.
            wk_sb = persist.tile([128, CK_CHUNKS, IN_PER_CORE], bf16, tag="wk")
            nc.sync.dma_start(out=wk_sb, in_=wk.rearrange("(k p) n -> p k n", p=128))
            wq_sb = persist.tile([128, QK_CHUNKS, IN_PER_CORE], bf16, tag="wq")
            nc.sync.dma_start(out=wq_sb, in_=wq.rearrange("(k p) n -> p k n", p=128))

            ctx_sb = persist.tile([128, CK_CHUNKS, M], bf16, tag="ctx")
            x0_sb = stream.tile([128, QK_CHUNKS, QB], bf16, tag="x", bufs=1,
                                name="x0")
            for k in range(QK_CHUNKS):
                if k < CK_CHUNKS:
                    nc.sync.dma_start(
                        out=ctx_sb[:, k, :], in_=cT[ds(k * 128, 128), :]
                    )
                nc.sync.dma_start(
                    out=x0_sb[:, k, :], in_=xT[ds(k * 128, 128), 0:QB]
                )
            wv_sb = persist.tile([128, CK_CHUNKS, IN_PER_CORE], bf16, tag="wv")
            nc.sync.dma_start(out=wv_sb, in_=wv.rearrange("(k p) n -> p k n", p=128))
            wo_sb = persist.tile([128, IN_CHUNKS, Q_DIM], bf16, tag="wo")
            nc.sync.dma_start(out=wo_sb, in_=wo.rearrange("(t p) n -> p t n", p=128))
            x1_sb = stream.tile([128, QK_CHUNKS, QB], bf16, tag="x2", bufs=1,
                                name="x1")
            nc.sync.dma_start(
                out=x1_sb,
                in_=xT[:, ds(QB, QB)].rearrange("(k p) q -> p k q", p=128),
            )
            x_tiles = [x0_sb, x1_sb]

            # ---- constants: PE-transpose identity + fp32 ones scratch for
            # the V ones-column (denominator trick).
            ident = persist.tile([128, 128], bf16, tag="ident")
            masks.make_identity(nc, ident)
            onesf = persist.tile([128, KC * H_PER_CORE], f32, tag="onesf")
            nc.vector.memset(onesf, 1.0)

            # ---- persistent tensors
            kT_sb = persist.tile([128, IN_CHUNKS, M], bf16, tag="kt")
            qT_tiles = [
                stream.tile([128, IN_CHUNKS, QB], bf16, tag="qt", bufs=1,
                            name="qT0"),
                stream.tile([128, IN_CHUNKS, QB], bf16, tag="qt2", bufs=1,
                            name="qT1"),
            ]
            # v chunks [keys, h, d+1]; col d holds 1.0 (denominator column)
            v_sb = persist.tile([128, KC, H_PER_CORE, D + 1], bf16, tag="v")
            nc.vector.tensor_copy(
                out=v_sb[:, :, :, D : D + 1],
                in_=onesf.rearrange("p (a b c) -> p a b c", a=KC, b=H_PER_CORE),
            )
            # probabilities for a whole head (16 key-chunks), ring by parity
            pt_tiles = [
                stream.tile([128, KC, QB], bf16, tag="pt0", bufs=1, name="ptA"),
                stream.tile([128, KC, QB], bf16, tag="pt1", bufs=1, name="ptB"),
            ]
            # normalized attention out (query-major), ring by parity
            otn_tiles = [
                stream.tile([128, QC, D], bf16, tag="otn0", bufs=1, name="otnA"),
                stream.tile([128, QC, D], bf16, tag="otn1", bufs=1, name="otnB"),
            ]
            # per-qc reciprocal denominators
            rc_tiles = [
                stream.tile([128, QC, 1], f32, tag="rc0", bufs=1, name="rcA"),
                stream.tile([128, QC, 1], f32, tag="rc1", bufs=1, name="rcB"),
            ]
            # assembled [inner, q] blocks for the final projection
            ot_all_tiles = [
                stream.tile([128, IN_CHUNKS, QB], bf16, tag="ota0", bufs=1,
                            name="otall0"),
                stream.tile([128, IN_CHUNKS, QB], bf16, tag="ota1", bufs=1,
                            name="otall1"),
            ]

            # ---- emission helpers ------------------------------------
            proj_psum = {}

            # kT(m) for one contraction chunk k: 4 matmuls into a whole-half
            # psA tile pair (used only during startup for m=0).  The half-1
            # copy is deferred (emit_kt_copy) so the DVE prioritizes the
            # qT copies that gate the first score chunk.
            def kt_startup(m, k):
                for half in range(2):
                    key = ("K", m, half)
                    if k == 0:
                        proj_psum[key] = psA.tile(
                            [128, QB], f32, tag="A", name=f"kps{m}{half}"
                        )
                    kps = proj_psum[key]
                    for j in range(2):
                        nc.tensor.matmul(
                            kps[:, ts(j, 512)],
                            wk_sb[:, k, ts(m, 128)],
                            ctx_sb[:, k, ds(half * 1024 + j * 512, 512)],
                            start=(k == 0),
                            stop=(k == CK_CHUNKS - 1),
                        )
                    if k == CK_CHUNKS - 1 and half == 0:
                        # ACT is idle before the first exp
                        nc.scalar.copy(
                            out=kT_sb[:, m, ds(half * 1024, 1024)], in_=kps
                        )

            def kt_startup_copy1(m):
                nc.vector.tensor_copy(
                    out=kT_sb[:, m, ds(1024, 1024)], in_=proj_psum[("K", m, 1)]
                )

            # kT(m=1) half-0 also rides the startup DMA window (psB ring);
            # its copies are deferred so the qT copies hit the DVE first.
            def kt_m1h0_startup(k):
                for j in range(2):
                    key = ("k", 1, 0, j)
                    if k == 0:
                        proj_psum[key] = psB.tile(
                            [128, 512], f32, tag="B", name=f"kps10{j}"
                        )
                    nc.tensor.matmul(
                        proj_psum[key],
                        wk_sb[:, k, ts(1, 128)],
                        ctx_sb[:, k, ds(j * 512, 512)],
                        start=(k == 0),
                        stop=(k == CK_CHUNKS - 1),
                    )

            def kt_m1h0_copies():
                for j in range(2):
                    nc.vector.tensor_copy(
                        out=kT_sb[:, 1, ds(j * 512, 512)],
                        in_=proj_psum[("k", 1, 0, j)],
                    )

            # j-split kT part for the filler path (1-bank psB psums)
            def kt_part(m, half, j, k0, kn):
                def f():
                    key = ("k", m, half, j)
                    if k0 == 0:
                        proj_psum[key] = psB.tile(
                            [128, 512], f32, tag="B", name=f"kps{m}{half}{j}"
                        )
                    kps = proj_psum[key]
                    for k in range(k0, kn):
                        nc.tensor.matmul(
                            kps,
                            wk_sb[:, k, ts(m, 128)],
                            ctx_sb[:, k, ds(half * 1024 + j * 512, 512)],
                            start=(k == 0),
                            stop=(k == CK_CHUNKS - 1),
                        )
                    if kn == CK_CHUNKS:
                        nc.vector.tensor_copy(
                            out=kT_sb[:, m, ds(half * 1024 + j * 512, 512)],
                            in_=kps,
                        )
                return f

            def kt_parts(m):
                return [kt_part(m, half, j, k0, k0 + 2)
                        for half in range(2) for j in range(2)
                        for k0 in (0, 2, 4)]

            def qt_startup(qb, m, k):
                # psums come from the (startup-idle) pvps ring so psB stays
                # free for the kT(m=1) head-0 accumulators.
                for j in range(2):
                    key = ("Q", qb, m, j)
                    if k == 0:
                        proj_psum[key] = pvps.tile(
                            [128, 512], f32, tag="PV", name=f"qps{qb}{m}{j}"
                        )
                    qps = proj_psum[key]
                    nc.tensor.matmul(
                        qps,
                        wq_sb[:, k, ts(m, 128)],
                        x_tiles[qb][:, k, ts(j, 512)],
                        start=(k == 0),
                        stop=(k == QK_CHUNKS - 1),
                    )
                    if k == QK_CHUNKS - 1:
                        eng = nc.scalar.copy if j == 0 else nc.vector.tensor_copy
                        eng(out=qT_tiles[qb][:, m, ds(j * 512, 512)], in_=qps)

            def qt_part(qb, m, j, k0, kn):
                def f():
                    key = ("Q", qb, m, j)
                    if k0 == 0:
                        proj_psum[key] = psB.tile(
                            [128, 512], f32, tag="B", name=f"qps{qb}{m}{j}"
                        )
                    qps = proj_psum[key]
                    for k in range(k0, kn):
                        nc.tensor.matmul(
                            qps,
                            wq_sb[:, k, ts(m, 128)],
                            x_tiles[qb][:, k, ts(j, 512)],
                            start=(k == 0),
                            stop=(k == QK_CHUNKS - 1),
                        )
                    if kn == QK_CHUNKS:
                        nc.vector.tensor_copy(
                            out=qT_tiles[qb][:, m, ds(j * 512, 512)], in_=qps
                        )
                return f

            def qt_parts(qb, m):
                return [qt_part(qb, m, j, k0, k0 + 2)
                        for j in range(2) for k0 in (0, 2, 4, 6)]

            def v_part(kc, k0, kn):
                def f():
                    key = ("v", kc)
                    if k0 == 0:
                        proj_psum[key] = psB.tile(
                            [128, IN_PER_CORE], f32, tag="B", name=f"vps{kc}"
                        )
                    vps = proj_psum[key]
                    for k in range(k0, kn):
                        nc.tensor.matmul(
                            vps,
                            ctx_sb[:, k, ts(kc, 128)],
                            wv_sb[:, k, :],
                            start=(k == 0),
                            stop=(k == CK_CHUNKS - 1),
                        )
                    if kn == CK_CHUNKS:
                        nc.vector.tensor_copy(
                            out=v_sb[:, kc, :, 0:D],
                            in_=vps.rearrange("p (h d) -> p h d", h=H_PER_CORE),
                        )
                return f

            def emit_v_chunk(kc):
                v_part(kc, 0, CK_CHUNKS)()

            # PV for one (head, query-chunk) into group psum `pv` slot qi.
            # kc0/kcn select a key sub-range so the reduction can be split
            # across the phase boundary (first half runs as soon as half the
            # exps are done).
            def emit_pv(hb, h, qi, qc, pv, kc0=0, kcn=KC):
                pt = pt_tiles[hb]
                for kc in range(kc0, kcn):
                    nc.tensor.matmul(
                        pv[:, qi, 0 : D + 1],
                        pt[:, kc, ts(qc, 128)],
                        v_sb[:, kc, h, :],
                        start=(kc == 0),
                        stop=(kc == KC - 1),
                    )

            # one reciprocal + one broadcast multiply per 4-chunk group
            def emit_norm_group(hb, g, pv):
                rc = rc_tiles[hb][:, 4 * g : 4 * g + 4, :]
                nc.vector.reciprocal(out=rc, in_=pv[:, :, D : D + 1])
                nc.vector.tensor_mul(
                    out=otn_tiles[hb][:, 4 * g : 4 * g + 4, :],
                    in0=pv[:, :, 0:D],
                    in1=rc.broadcast_to([128, 4, D]),
                )

            # PE transpose of one normalized [128q, D] chunk into the
            # assembly psum at partition base (h%2)*64, column qc*128.
            def emit_transpose(hb, tp, qc):
                po = hb * D
                nc.tensor.matmul(
                    tp[po : po + D, ts(qc, 128)],
                    otn_tiles[hb][:, qc, :],
                    ident,
                    is_transpose=True,
                )

            def emit_tp_copy(qb, t, tp, qc0=0, qcn=QC, on_act=False):
                eng = nc.scalar.copy if on_act else nc.vector.tensor_copy
                eng(
                    out=ot_all_tiles[qb][:, t, ds(qc0 * 128, (qcn - qc0) * 128)],
                    in_=tp[:, ds(qc0 * 128, (qcn - qc0) * 128)],
                )

            # one final-projection chunk: 128 queries x 1024 out-dims,
            # two j-half psums, copies split across engines, ONE output DMA
            # (the 625ns/DMA HWDGE prep is the drain bottleneck otherwise).
            def emit_final(qb, qm, split=False):
                ost = stream.tile([128, Q_DIM], bf16, tag="ost", bufs=4,
                                  name=f"ost{qb}{qm}")
                for j in range(2):
                    ops = psB.tile([128, 512], f32, tag="B",
                                   name=f"ops{qb}{qm}{j}")
                    for t in range(IN_CHUNKS):
                        nc.tensor.matmul(
                            ops,
                            ot_all_tiles[qb][:, t, ts(qm, 128)],
                            wo_sb[:, t, ts(j, 512)],
                            start=(t == 0),
                            stop=(t == IN_CHUNKS - 1),
                        )
                    if split and j == 0:
                        nc.scalar.copy(out=ost[:, ts(j, 512)], in_=ops)
                    else:
                        nc.vector.tensor_copy(out=ost[:, ts(j, 512)], in_=ops)
                nc.sync.dma_start(
                    out=out_d[ds(qb * QB + qm * 128, 128), :], in_=ost
                )

            # ---- attention stream ------------------------------------
            def run_phase(qb, h, fillers):
                hb = h % 2
                t, po = h // 2, (h % 2) * D
                qT_sb = qT_tiles[qb]
                for kc in range(KC):
                    st = psA.tile([128, QB], f32, tag="A", name=f"st{qb}{h}{kc}")
                    for j in range(2):
                        nc.tensor.matmul(
                            st[:, ts(j, 512)],
                            kT_sb[po : po + D, t, ts(kc, 128)],
                            qT_sb[po : po + D, t, ts(j, 512)],
                            start=True,
                            stop=True,
                        )
                    nc.scalar.activation(
                        out=pt_tiles[hb][:, kc, :], in_=st, func=Exp, scale=SCALE
                    )
                    for f in fillers.get(kc, ()):
                        f()

            pv_live = {}

            def mk_pv_phase(prev_h, kc0=6):
                # PV of the previous head at kc0..kc0+7, one query-chunk per
                # slot; psums alternate between two 4-chunk group tiles and
                # each group is normalized right after its last chunk.
                # Starting at kc 6 leaves two slots of slack so the NEXT
                # phase's first exp never waits on this head's pt reads.
                prev_hb = prev_h % 2
                def pv_qc(qc):
                    def f():
                        g, qi = qc // 4, qc % 4
                        if qi == 0:
                            pv_live[g] = pvps.tile(
                                [128, 4, 128], f32, tag="PV",
                                name=f"pv{prev_h}{g}"
                            )
                        emit_pv(prev_hb, prev_h, qi, qc, pv_live[g])
                        if qi == 3:
                            emit_norm_group(prev_hb, g, pv_live[g])
                    return f
                return {kc0 + qc: [pv_qc(qc)] for qc in range(QC)}

            def merge(*dicts):
                out = {}
                for d in dicts:
                    for k, v in d.items():
                        out.setdefault(k, []).extend(v)
                return out

            def sched_list(items, kc0, kcn):
                slots = {}
                n = kcn - kc0
                per = (len(items) + n - 1) // n
                it = iter(items)
                for kc in range(kc0, kcn):
                    for _ in range(per):
                        f = next(it, None)
                        if f is not None:
                            slots.setdefault(kc, []).append(f)
                return slots

            tp_tiles = {}

            def mk_transposes(tr_h, kc0=0, kcn=4, qc0=0, qcn=QC):
                hb = tr_h % 2
                t = tr_h // 2
                def tp_qc(qc):
                    def f():
                        if qc == 0 and hb == 0:
                            tp_tiles[t % 2] = psB.tile(
                                [128, QB], bf16, tag="B", name=f"tp{tr_h}"
                            )
                        emit_transpose(hb, tp_tiles[t % 2], qc)
                    return f
                return sched_list([tp_qc(q) for q in range(qc0, qcn)], kc0, kcn)

            def mk_copy(qb, t, kc):
                return {kc: [lambda: emit_tp_copy(qb, t, tp_tiles[t % 2])]}

            # ---------------- startup: kT(m=0) + qT0(m=0) ride the DMAs
            for k in range(QK_CHUNKS):
                if k < CK_CHUNKS:
                    kt_startup(0, k)
                    kt_m1h0_startup(k)
                qt_startup(0, 0, k)
            kt_startup_copy1(0)
            kt_m1h0_copies()

            # ---------------- qb 0 ----------------
            run_phase(0, 0, merge(
                sched_list(qt_parts(0, 1), 0, 8),
                sched_list([lambda kc=kc: emit_v_chunk(kc) for kc in range(8)],
                           8, 16),
            ))
            run_phase(0, 1, merge(
                sched_list([v_part(kc, k0, k0 + 3)
                            for kc in range(8, KC) for k0 in (0, 3)], 0, 6),
                mk_pv_phase(0),
                sched_list([kt_part(1, 1, j, k0, k0 + 2)
                            for j in range(2) for k0 in (0, 2, 4)], 8, 14),
            ))
            run_phase(0, 2, merge(
                mk_transposes(0, 0, 4),
                mk_pv_phase(1),
            ))
            run_phase(0, 3, merge(
                mk_transposes(1, 0, 4),
                mk_copy(0, 0, 4),
                sched_list(qt_parts(1, 0), 2, 6),
                mk_pv_phase(2),
            ))

            # ---------------- qb 1 ----------------
            run_phase(1, 0, merge(
                mk_transposes(2, 0, 4),
                sched_list(qt_parts(1, 1), 2, 6),
                mk_pv_phase(3),
            ))
            fin0 = [lambda qm=qm: emit_final(0, qm) for qm in range(8)]
            run_phase(1, 1, merge(
                mk_transposes(3, 0, 4),
                mk_copy(0, 1, 4),
                sched_list(fin0[0:3], 5, 8),
                mk_pv_phase(0),
            ))
            run_phase(1, 2, merge(
                sched_list(fin0[3:8], 0, 5),
                mk_pv_phase(1),
            ))
            # phase 3: transposes of heads 0/1 (qb1) early, PV(2) mid,
            # head 2's transposes late (after its norm groups land).
            run_phase(1, 3, merge(
                mk_transposes(0, 0, 4),
                mk_transposes(1, 4, 8),
                mk_copy(1, 0, 8),
                mk_pv_phase(2),
                mk_transposes(2, 10, 12, 0, 4),
                mk_transposes(2, 14, 16, 4, 8),
            ))

            # ---------------- epilogue ----------------
            # head 3's PV (two 4-chunk group tiles, sequential groups), then
            # normalize / transpose / assemble / project.  Finals for qm 0-3
            # only need the first half of the t=1 block, so they start as
            # soon as the first 4 transposes are copied.
            pv3 = {}
            for g in range(2):
                pv3[g] = pvps.tile([128, 4, 128], f32, tag="PV", name=f"pv3{g}")
                for qi in range(4):
                    emit_pv(1, 3, qi, 4 * g + qi, pv3[g])
                emit_norm_group(1, g, pv3[g])
            for qc in range(4):
                emit_transpose(1, tp_tiles[1], qc)  # head 3 -> partitions 64:128
            emit_tp_copy(1, 1, tp_tiles[1], 0, 4, on_act=True)
            for qm in range(4):
                emit_final(1, qm, split=True)
            for qc in range(4, QC):
                emit_transpose(1, tp_tiles[1], qc)
            emit_tp_copy(1, 1, tp_tiles[1], 4, QC)
            for qm in range(4, QC):
                emit_final(1, qm, split=True)

    nc.finalize()
    return nc


def _get_nc():
    global _CACHED_NC
    if _CACHED_NC is None:
        _CACHED_NC = _build_bass()
    return _CACHED_NC


def _numpy_fallback(x, context, mask, Wq, Wk, Wv, Wout, bout):
    q = (x @ Wq.T).reshape(B, N, H, D)
    k = (context @ Wk.T).reshape(B, M, H, D)
    v = (context @ Wv.T).reshape(B, M, H, D)
    sim = np.einsum("bnhd,bmhd->bhnm", q, k) * SCALE
    sim = np.where(mask[:, None, None, :], sim, -np.finfo(np.float32).max)
    sim -= sim.max(axis=-1, keepdims=True)
    attn = np.exp(sim)
    attn /= attn.sum(axis=-1, keepdims=True)
    out = np.einsum("bhnm,bmhd->bnhd", attn, v).reshape(B, N, INNER)
    return (out @ Wout.T + bout).astype(np.float32)


def kernel(x, context, mask, Wq, Wk, Wv, Wout, bout, _want_results=False):
    import ml_dtypes

    bf = ml_dtypes.bfloat16
    x = np.asarray(x, dtype=np.float32)
    context = np.asarray(context, dtype=np.float32)
    mask = np.asarray(mask)
    Wq = np.asarray(Wq, dtype=np.float32)
    Wk = np.asarray(Wk, dtype=np.float32)
    Wv = np.asarray(Wv, dtype=np.float32)
    Wout = np.asarray(Wout, dtype=np.float32)
    bout = np.asarray(bout, dtype=np.float32)

    if not mask.all():
        return _numpy_fallback(x, context, mask, Wq, Wk, Wv, Wout, bout)

    from concourse.bass_utils import run_bass_kernel_spmd

    in_maps = []
    for c in range(N_CORES):
        b, hg = c // 2, c % 2
        sl = slice(hg * IN_PER_CORE, (hg + 1) * IN_PER_CORE)
        in_maps.append(
            {
                "xT": np.ascontiguousarray(x[b].T).astype(bf),
                "cT": np.ascontiguousarray(context[b].T).astype(bf),
                "wq": np.ascontiguousarray(Wq[sl, :].T).astype(bf),
                "wk": np.ascontiguousarray(Wk[sl, :].T).astype(bf),
                "wv": np.ascontiguousarray(Wv[sl, :].T).astype(bf),
                "wo": np.ascontiguousarray(Wout[:, sl].T).astype(bf),
            }
        )

    res = run_bass_kernel_spmd(_get_nc(), in_maps, core_ids=list(range(N_CORES)))

    out = np.empty((B, N, Q_DIM), dtype=np.float32)
    for b in range(B):
        out[b] = (
            res.results[2 * b]["out"].astype(np.float32)
            + res.results[2 * b + 1]["out"].astype(np.float32)
            + bout
        )
    if _want_results:
        return out, res
    return out
